# revision 30
# baseline (speedup 1.0000x reference)
"""Trainium2 Bass kernel for nn_BaseTransformer (ensemble member-attention block).

Sharding: data-parallel over batch B=8 across 8 NeuronCores (1 batch each).
Weights/constants replicated. No collectives.

Reference math (per batch b, x = in_tensor[b] as [K=16, C=64, S=4096]):
  value = einsum('ics,oc->ios', x, Wv)
  key   = selu(einsum(x, Wk)); query = selu(einsum(x, Wq))
  gram[c,i,j] = sum_s key[i,c,s] query[j,c,s] / 64        (then * lambda^2 fold)
  A = softmax(gram, axis=i) + I
  transformed[j] = sum_i (A[c,i,j] - 1/16) value_i        (exact mean fold)
  out = selu(x + einsum(transformed, w_out) + b_out)

v10 layout/DMA scheme (1126us baseline -> ~387us):
  - x loaded via SWDGE (gpsimd) DMAs with fp32->bf16 cast-in-flight, in two
    s-halves of 16 half tiles [128, 2048] (members t, t+8 stacked).
  - all SBUF->DRAM stores ride SWDGE (descriptors spread over all 16 SDMA
    engines; dynamic HWDGE stores funnel through engines 0/1). Loads stay on
    the sync/scalar HWDGE queues (they spray fine).
  - phase-2 scratch round trips use ONE merged strided DMA per direction per
    chunk (3-dim APs) instead of 8-16 small DMAs.
  - residual add is a PE identity-matmul accumulate from resident bf16 x;
    b_out is folded into the ACT bias of the output selu; output is stored
    bf16 and upcast on host.
  - stride-8 head groups (head c = 8u+g); value/gram q-layout q=64*i2+8u+it
    (pi_in, = baseline) and mix output layout p=64*j2+8*jt+u (pi_out,
    u-innermost) so the mix->outconv regroup is a 3-dim AP.
  - selu(t) = min(alpha*e^t - alpha, relu(t)) composed from ACT Exp,
    ACT Relu, DVE scalar_tensor_tensor.
  - phase 1 software-pipelined (gram skewed one chunk behind kq so the PE
    never waits on the selu chain; value conv spread through the phase);
    phase 2 skewed gather | tload | mix+out so the DRAM round trips hide.
  - kq/kqT storage is g-major (hs = 8g+u <-> head 8u+g) so gram matmul
    operands are contiguous slices (stride-8 APs halve PE throughput).
"""

import sys

if "/opt/trn_rl_repo" not in sys.path:
    sys.path.insert(0, "/opt/trn_rl_repo")

import numpy as np

import concourse.bass as bass
import concourse.bacc as bacc
import concourse.mybir as mybir
import concourse.tile as tile

F32 = mybir.dt.float32
BF16 = mybir.dt.bfloat16

K, C, HEADS, S = 16, 64, 64, 4096
NG = 8           # head groups of 8 (stride-8: group g = heads {8u+g})
SC1 = 128        # phase-1 spatial chunk (gram contraction tile)
NCH1 = S // SC1  # 32
SC2 = 512        # phase-2 spatial chunk
NCH2 = S // SC2  # 8
SH = 1024        # x load quarter
NCH1H = SH // SC1  # 16 phase-1 chunks per half
NCH2H = SH // SC2  # 4 phase-2 chunks per half

ALPHA = 1.6732632423543772
LAMBDA = 1.0507009873554805
LN_ALPHA = float(np.log(ALPHA))
LN_LAMBDA_ALPHA = float(np.log(LAMBDA * ALPHA))
GRAM_SCALE = float(LAMBDA * LAMBDA / 64.0)


def _pi_in(u, i):
    return 64 * (i // 8) + 8 * u + (i % 8)


def _pi_out(u, j):
    return 64 * (j // 8) + 8 * (j % 8) + u


def host_constants(w_value, w_key, w_query, w_out, b_out):
    """Build all replicated device inputs on the host."""
    consts = {}
    # sigma: head c = 8u+g  <->  storage position 8g+u (group-contiguous).
    sigma = np.zeros(64, np.int64)
    for u in range(8):
        for g in range(8):
            sigma[8 * g + u] = 8 * u + g
    wvT = np.ascontiguousarray(w_value.T[:, sigma])
    consts["wvT"] = np.concatenate([wvT, wvT], axis=0).astype(np.float32)
    # k/q head storage is g-major (hs = 8g+u <-> real head 8u+g) so the
    # gram operand APs are contiguous 8-runs instead of stride-8.
    wkqT = np.ascontiguousarray(np.concatenate(
        [w_key.T[:, sigma], w_query.T[:, sigma]], axis=1))
    consts["wkqT"] = np.concatenate([wkqT, wkqT], axis=0).astype(np.float32)

    # Block-diag out-conv weight: W2[(j2', u, g), (j2, o)] = [j2'==j2] *
    # w_out[o, 8u+g]  (tload tile rows are (j2, u, g); po rows are (j2, o)).
    W2 = np.zeros((128, 128), np.float32)
    for j2 in range(2):
        for u in range(8):
            for g in range(8):
                W2[64 * j2 + 8 * u + g, 64 * j2: 64 * (j2 + 1)] = \
                    w_out[:, 8 * u + g]
    consts["W2"] = W2

    # Gram psum layout: partition = 8j+u (q side), free = 8i+u' (k side).
    # MASK zeroes cross-head entries (u != u').
    mask = np.zeros((128, 128), np.float32)
    for p in range(128):
        for f in range(128):
            if p % 8 == f % 8:
                mask[p, f] = 1.0
    consts["maskg"] = mask

    # permP (mm2 lhsT): rows r=(i,u)=8i+u -> bigB row pi_in(u, i).
    P = np.zeros((128, 128), np.float32)
    for u in range(8):
        for i in range(16):
            P[8 * i + u, _pi_in(u, i)] = 1.0
    consts["permP"] = P
    # permPp (mm1 rhs): rows r=(j,u)=8j+u -> bigB column pi_out(u, j).
    Pp = np.zeros((128, 128), np.float32)
    for u in range(8):
        for j in range(16):
            Pp[8 * j + u, _pi_out(u, j)] = 1.0
    consts["permPp"] = Pp

    # DPAT: D[pi_in(u,i), pi_out(u,j)] = delta(i,j) - 1/16.
    D = np.zeros((128, 128), np.float32)
    for u in range(8):
        for i in range(16):
            for j in range(16):
                D[_pi_in(u, i), _pi_out(u, j)] = \
                    (1.0 if i == j else 0.0) - 1.0 / 16.0
    consts["dpat"] = D

    consts["ident"] = np.eye(128, dtype=np.float32)
    # b_out folded into the output selu ACT biases; po rows are (j2, o).
    b2 = np.concatenate([b_out, b_out]).astype(np.float32)
    consts["bo_exp"] = (b2 + LN_LAMBDA_ALPHA).reshape(128, 1)
    consts["bo_relu"] = (b2 * LAMBDA).reshape(128, 1)
    return consts


def build_nc():
    """Build the single-core Bass program (same NEFF on all 8 cores)."""
    nc = bacc.Bacc("TRN2", target_bir_lowering=False, debug=False)

    x_d = nc.dram_tensor("x", [K, C, S], F32, kind="ExternalInput")
    wvT_d = nc.dram_tensor("wvT", [128, 64], F32, kind="ExternalInput")
    wkqT_d = nc.dram_tensor("wkqT", [128, 128], F32, kind="ExternalInput")
    W2_d = nc.dram_tensor("W2", [128, 128], F32, kind="ExternalInput")
    mask_d = nc.dram_tensor("maskg", [128, 128], F32, kind="ExternalInput")
    permP_d = nc.dram_tensor("permP", [128, 128], F32, kind="ExternalInput")
    permPp_d = nc.dram_tensor("permPp", [128, 128], F32, kind="ExternalInput")
    dpat_d = nc.dram_tensor("dpat", [128, 128], F32, kind="ExternalInput")
    ident_d = nc.dram_tensor("ident", [128, 128], F32, kind="ExternalInput")
    boe_d = nc.dram_tensor("bo_exp", [128, 1], F32, kind="ExternalInput")
    bor_d = nc.dram_tensor("bo_relu", [128, 1], F32, kind="ExternalInput")
    out_d = nc.dram_tensor("out", [K, C, S], BF16, kind="ExternalOutput")

    # DRAM scratch for the partition regroups (DRAM-side APs may stride).
    # vscr[pc][g][q=64*i2+8u+it][s]   (value, member-permuted for the mix)
    # mscr[pc][jt][(j2,u,g)][s]       (mix out, regrouped for the out conv)
    vscr_d = nc.dram_tensor("vscr", [NCH2, NG, 128, SC2], BF16)
    mscr_d = nc.dram_tensor("mscr", [NCH2, 8, 128, SC2], BF16)

    with tile.TileContext(nc) as tc:
        with (
            tc.tile_pool(name="persist", bufs=1) as persist,
            tc.tile_pool(name="xpool", bufs=1) as xpool,
        ):
            # ---- weights / constants to SBUF (+ bf16 casts) ----
            wv_f = persist.tile([128, 64], F32, tag="wvf")
            nc.scalar.dma_start(out=wv_f, in_=wvT_d[:, :])
            wv_sb = persist.tile([128, 64], BF16, tag="wv")
            nc.vector.tensor_copy(wv_sb, wv_f)
            wkq_f = persist.tile([128, 128], F32, tag="wkqf")
            nc.scalar.dma_start(out=wkq_f, in_=wkqT_d[:, :])
            wkq_sb = persist.tile([128, 128], BF16, tag="wkq")
            nc.vector.tensor_copy(wkq_sb, wkq_f)
            W2_f = persist.tile([128, 128], F32, tag="W2f")
            nc.scalar.dma_start(out=W2_f, in_=W2_d[:, :])
            W2_sb = persist.tile([128, 128], BF16, tag="W2")
            nc.vector.tensor_copy(W2_sb, W2_f)
            id_f = persist.tile([128, 128], F32, tag="idf")
            nc.scalar.dma_start(out=id_f, in_=ident_d[:, :])
            id_sb = persist.tile([128, 128], BF16, tag="ident")
            nc.vector.tensor_copy(id_sb, id_f)
            mask_sb = persist.tile([128, 128], F32, tag="mask")
            nc.scalar.dma_start(out=mask_sb, in_=mask_d[:, :])
            permP_sb = persist.tile([128, 128], F32, tag="permP")
            nc.scalar.dma_start(out=permP_sb, in_=permP_d[:, :])
            permPp_sb = persist.tile([128, 128], F32, tag="permPp")
            nc.scalar.dma_start(out=permPp_sb, in_=permPp_d[:, :])
            dpat_sb = persist.tile([128, 128], F32, tag="dpat")
            nc.scalar.dma_start(out=dpat_sb, in_=dpat_d[:, :])
            boe_sb = persist.tile([128, 1], F32, tag="boe")
            nc.scalar.dma_start(out=boe_sb, in_=boe_d[:, :])
            bor_sb = persist.tile([128, 1], F32, tag="bor")
            nc.scalar.dma_start(out=bor_sb, in_=bor_d[:, :])
            lna_sb = persist.tile([128, 1], F32, tag="lna")
            nc.vector.memset(lna_sb, LN_ALPHA)
            zero_sb = persist.tile([128, 1], F32, tag="zero")
            nc.vector.memset(zero_sb, 0.0)

            # ---- x: SWDGE casting loads into 16 half tiles [128, 2048] ----
            # x_sb[t][h] holds members (t, t+8), s-range [2048h, 2048h+2048).
            xv_d = x_d.rearrange("(m2 t) c s -> t m2 c s", m2=2, t=8)
            x_sb = [[None] * (S // SH) for _ in range(8)]
            for h in range(S // SH):
                ssl = slice(SH * h, SH * (h + 1))
                for t in range(8):
                    xb = xpool.tile([128, SH], BF16, tag=f"x{t}h{h}")
                    nc.gpsimd.dma_start(out=xb, in_=xv_d[t, :, :, ssl])
                    x_sb[t][h] = xb

            # BigB result tiles (persist into phase 2), bf16 for the mix
            bigB = []
            for g in range(NG):
                bigB_t = persist.tile([128, 128], BF16, tag=f"bigB{g}")
                bigB.append(bigB_t)

            vstores = {}  # pc -> [store instrs]

            # =========================== PHASE 1 ===========================
            # vscr store view: per (pc, i2): [64 p=(g,u)][(t, s) 4096]
            #   dst addr = g*65536 + (64*i2 + 8u + t)*512 + s
            vdst = vscr_d.rearrange(
                "pc g (i2 u it) s -> pc i2 g u (it s)", i2=2, u=8, it=8)
            with (
                tc.tile_pool(name="p1sb", bufs=4) as p1sb,
                tc.tile_pool(name="p1sc", bufs=4) as p1sc,
                tc.tile_pool(name="vall", bufs=3) as vallp,
                tc.tile_pool(name="kqps", bufs=2, space="PSUM") as kqps,
                tc.tile_pool(name="vps", bufs=2, space="PSUM") as vps,
                tc.tile_pool(name="gramps", bufs=1, space="PSUM") as gramps,
            ):
                gram_ps = []
                for gb in range(2):
                    gram_t = gramps.tile([128, 512], F32, tag=f"gram{gb}")
                    gram_ps.append(gram_t)

                kqT_live = {}

                def kq_part(sc):
                    h = sc // NCH1H
                    sl = slice(SC1 * (sc % NCH1H), SC1 * (sc % NCH1H + 1))
                    # kqT free layout: half*1024 + m*64 + c   (bf16)
                    kqT = p1sb.tile([128, K * 128], BF16, tag="kqT")
                    kqT_live[sc] = kqT
                    for blk in range(2):  # member blocks [0..8), [8..16)
                        ps = kqps.tile([128, 8 * 128], F32, tag="kqps")
                        for mb in range(8):
                            m = blk * 8 + mb
                            xt = x_sb[m % 8][h]
                            rhalf = slice(0, 64) if m < 8 else slice(64, 128)
                            nc.tensor.matmul(
                                ps[:, 128 * mb: 128 * (mb + 1)],
                                xt[rhalf, sl], wkq_sb[rhalf, :],
                                start=True, stop=True,
                            )
                        # selu: e2 = exp(kq + ln a); r = relu(kq);
                        # out = (e2 - a) min r   (all bf16 outputs).
                        # e2/r free layout: half*512 + g*64 + mb*8 + u
                        # (g-major, matching the g-major kqT target).
                        e2 = p1sc.tile([128, 8 * 128], BF16, tag="e2")
                        e2v = e2.rearrange("p (h g mb u) -> p mb (h g) u",
                                           h=2, g=8, mb=8, u=8)
                        nc.scalar.activation(
                            out=e2v, in_=ps,
                            func=mybir.ActivationFunctionType.Exp,
                            bias=lna_sb[:, 0:1])
                        r = p1sc.tile([128, 8 * 128], BF16, tag="r")
                        rv = r.rearrange("p (h g mb u) -> p mb (h g) u",
                                         h=2, g=8, mb=8, u=8)
                        nc.scalar.activation(
                            out=rv, in_=ps,
                            func=mybir.ActivationFunctionType.Relu,
                            bias=zero_sb[:, 0:1])
                        stt_eng = nc.vector
                        # kqT free layout: half*1024 + g*128 + m*8 + u
                        # (g-major so the gram operands are contiguous).
                        kqw = kqT.rearrange(
                            "p (hh gg m2 f) -> p hh m2 gg f",
                            hh=2, gg=8, m2=2, f=64)
                        for half in range(2):
                            stt_eng.scalar_tensor_tensor(
                                out=kqw[:, half, blk],
                                in0=e2[:, 512 * half: 512 * (half + 1)],
                                scalar=ALPHA,
                                in1=r[:, 512 * half: 512 * (half + 1)],
                                op0=mybir.AluOpType.subtract,
                                op1=mybir.AluOpType.min)
                def gram_part(sc):
                    # gram: lhsT = q side (M = 8j+u), rhs = k side (N = 8i+u')
                    kqT = kqT_live.pop(sc)
                    vq = kqT.rearrange("p (hh gg f) -> p hh gg f",
                                       hh=2, gg=8, f=128)
                    for g in range(NG):
                        q_ap = vq[:, 1, g]
                        k_ap = vq[:, 0, g]
                        nc.tensor.matmul(
                            gram_ps[g // 4][:, 128 * (g % 4): 128 * (g % 4 + 1)],
                            q_ap, k_ap,
                            start=(sc == 0 and g % 4 == 0),
                            stop=(sc == NCH1 - 1 and g % 4 == 3))

                vall_live = {}

                def value_units(pc, ts):
                    h = pc // NCH2H
                    sl = slice(SC2 * (pc % NCH2H), SC2 * (pc % NCH2H + 1))
                    if pc not in vall_live:
                        vall_new = vallp.tile([128, 8 * SC2], BF16, tag="vall")
                        vall_live[pc] = vall_new
                    vall = vall_live[pc]
                    for t in ts:
                        ps = vps.tile([128, SC2], F32, tag="vps")
                        nc.tensor.matmul(
                            ps[0:64, :], wv_sb[0:64, :], x_sb[t][h][0:64, sl],
                            start=True, stop=True)
                        nc.tensor.matmul(
                            ps[64:128, :], wv_sb[64:128, :],
                            x_sb[t][h][64:128, sl],
                            start=True, stop=True)
                        nc.vector.tensor_copy(
                            vall[:, SC2 * t: SC2 * (t + 1)], ps)
                    if ts[-1] == 7:
                        vall = vall_live.pop(pc)
                        stores = []
                        for i2 in range(2):
                            sti = nc.gpsimd.dma_start(
                                out=vdst[pc, i2],
                                in_=vall[64 * i2: 64 * (i2 + 1), :])
                            stores.append(sti)
                        vstores[pc] = stores

                # Skew gram one chunk behind kq so the PE never waits on the
                # DVE/ACT selu chain (kq(sc+1) fills the gap), and sprinkle
                # value chunks in to keep the PE warm.
                # value-unit schedule: front-loaded (3/chunk after sc 8)
                # so value finishes with the gram; x is fully resident by
                # the time any unit needs a later quarter.
                vsched = [[] for _ in range(NCH1)]
                nxt = 0
                for sc in range(NCH1):
                    take = 2 if sc < 8 else 3
                    for _ in range(take):
                        if nxt < 64:
                            vsched[sc].append((nxt // 8, nxt % 8))
                            nxt += 1
                for sc in range(NCH1):
                    kq_part(sc)
                    if sc >= 1:
                        gram_part(sc - 1)
                    for pc, t in vsched[sc]:
                        value_units(pc, [t])
                gram_part(NCH1 - 1)

                # ---- softmax (no max-sub; range pre-verified) + BigB ----
                for g in range(NG):
                    gp = gram_ps[g // 4][:, 128 * (g % 4): 128 * (g % 4 + 1)]
                    E = p1sc.tile([128, 128], F32, tag="E")
                    nc.scalar.activation(
                        out=E, in_=gp,
                        func=mybir.ActivationFunctionType.Exp,
                        bias=zero_sb[:, 0:1], scale=GRAM_SCALE)
                    Ssum = p1sc.tile([128, 8], F32, tag="Ssum")
                    nc.vector.tensor_reduce(
                        out=Ssum,
                        in_=E.rearrange("p (i u) -> p u i", i=16, u=8),
                        axis=mybir.AxisListType.X, op=mybir.AluOpType.add)
                    R = p1sc.tile([128, 8], F32, tag="R")
                    nc.vector.reciprocal(out=R, in_=Ssum)
                    # normalize all 8 u-slices in one op via a stride-0
                    # broadcast of R over the i axis
                    Eu2 = E.rearrange("p (i u) -> p i u", i=16, u=8)
                    R3 = R.rearrange("p (a u) -> p a u", a=1, u=8)
                    _, R3b = bass.broadcast_tensor_aps(Eu2, R3)
                    nc.vector.tensor_tensor(
                        out=Eu2, in0=Eu2, in1=R3b, op=mybir.AluOpType.mult)
                    nc.vector.tensor_tensor(
                        out=E, in0=E, in1=mask_sb, op=mybir.AluOpType.mult)
                    c_ps = kqps.tile([128, 128], F32, tag="kqps")
                    nc.tensor.matmul(c_ps, E, permPp_sb, start=True, stop=True)
                    c_sb = p1sc.tile([128, 128], F32, tag="permcsb")
                    nc.scalar.copy(c_sb, c_ps)
                    b_ps = kqps.tile([128, 128], F32, tag="kqps")
                    nc.tensor.matmul(b_ps, permP_sb, c_sb, start=True, stop=True)
                    nc.vector.scalar_tensor_tensor(
                        out=bigB[g], in0=b_ps, scalar=1.0, in1=dpat_sb,
                        op0=mybir.AluOpType.mult, op1=mybir.AluOpType.add)

            # =========================== PHASE 2 ===========================
            # Per chunk: merged vgather -> 8 mix matmuls -> merged mstore ->
            # merged tload -> per member-pair: out conv + identity-residual
            # matmul, selu via ACT Exp/Relu (b_out in bias) + DVE stt ->
            # merged bf16 outstore.
            vgat = vscr_d.rearrange("pc g q s -> pc q g s")
            # mscr store view: per (pc, j2): [64 p=(jt,u)][(g, s) 4096]
            mdst = mscr_d.rearrange(
                "pc jt (j2 u g) s -> pc j2 jt u (g s)", j2=2, u=8, g=8)
            mgat = mscr_d.rearrange("pc jt q s -> pc q jt s")
            # out store view: per (pc, j2): [64 p=o][(jt, s)]
            odst = out_d.rearrange("(j2 jt) c s -> j2 c jt s", j2=2, jt=8)
            with (
                tc.tile_pool(name="p2in", bufs=4) as p2in,
                tc.tile_pool(name="p2sc", bufs=3) as p2sc,
                tc.tile_pool(name="p2out", bufs=2) as p2outp,
                tc.tile_pool(name="mps", bufs=5, space="PSUM") as mps,
                tc.tile_pool(name="ops", bufs=3, space="PSUM") as ops,
            ):
                mstores = {}
                vgs = {}
                talls = {}

                def stage_gather(pc):
                    vg = p2in.tile([128, 8 * SC2], BF16, tag="vg")
                    gi = nc.sync.dma_start(out=vg, in_=vgat[pc])
                    for sti in vstores.pop(pc):
                        tile.add_dep_helper(
                            gi.ins, sti.ins, reason="vgather after vstores")
                    vgs[pc] = vg

                def stage_mixout(mp, op):
                    """Interleave mix units of chunk mp with out units of
                    chunk op so PE/ACT/DVE all stay fed."""
                    vg = vgs.pop(mp) if mp is not None else None
                    mall = None
                    if mp is not None:
                        mall = p2sc.tile([128, 8 * SC2], BF16, tag="mall")
                    if op is not None:
                        h = op // NCH2H
                        sl = slice(SC2 * (op % NCH2H), SC2 * (op % NCH2H + 1))
                        gsl = slice(SC2 * op, SC2 * (op + 1))
                        tall = talls.pop(op)
                        o_all = p2outp.tile([128, 8 * SC2], BF16, tag="oall")
                    for k in range(8):
                        if op is not None:
                            po = ops.tile([128, SC2], F32, tag="ops")
                            nc.tensor.matmul(
                                po, W2_sb, tall[:, SC2 * k: SC2 * (k + 1)],
                                start=True, stop=False)
                            nc.tensor.matmul(
                                po, id_sb, x_sb[k][h][:, sl],
                                start=False, stop=True)
                            e2f = p2sc.tile([128, SC2], BF16, tag="fe2")
                            nc.scalar.activation(
                                out=e2f, in_=po,
                                func=mybir.ActivationFunctionType.Exp,
                                bias=boe_sb[:, 0:1])
                            r2f = p2sc.tile([128, SC2], BF16, tag="fr2")
                            nc.scalar.activation(
                                out=r2f, in_=po,
                                func=mybir.ActivationFunctionType.Relu,
                                bias=bor_sb[:, 0:1], scale=LAMBDA)
                            nc.vector.scalar_tensor_tensor(
                                out=o_all[:, SC2 * k: SC2 * (k + 1)],
                                in0=e2f, scalar=float(LAMBDA * ALPHA),
                                in1=r2f,
                                op0=mybir.AluOpType.subtract,
                                op1=mybir.AluOpType.min)
                    for k in range(8):
                        if mp is not None:
                            pm = mps.tile([128, SC2], F32, tag="mps")
                            nc.tensor.matmul(
                                pm, bigB[k], vg[:, SC2 * k: SC2 * (k + 1)],
                                start=True, stop=True)
                            # drain copies alternate DVE/ACT: the mix phase
                            # is paced by these, and ACT is idle during it
                            if k % 2 == 0:
                                nc.vector.tensor_copy(
                                    mall[:, SC2 * k: SC2 * (k + 1)], pm)
                            else:
                                nc.scalar.copy(
                                    mall[:, SC2 * k: SC2 * (k + 1)], pm)
                    if mp is not None:
                        mst = []
                        for j2 in range(2):
                            si = nc.gpsimd.dma_start(
                                out=mdst[mp, j2],
                                in_=mall[64 * j2: 64 * (j2 + 1), :])
                            mst.append(si)
                        mstores[mp] = mst
                    if op is not None:
                        for j2 in range(2):
                            nc.gpsimd.dma_start(
                                out=odst[j2, :, :, gsl],
                                in_=o_all[64 * j2: 64 * (j2 + 1), :])

                def stage_tload(pc):
                    tall = p2in.tile([128, 8 * SC2], BF16, tag="tall")
                    li = nc.scalar.dma_start(out=tall, in_=mgat[pc])
                    for si in mstores.pop(pc):
                        tile.add_dep_helper(
                            li.ins, si.ins, reason="tload after mstores")
                    talls[pc] = tall

                # Skewed pipeline: gather(pc) | tload(pc-2) | interleaved
                # [mix(pc-1) + out(pc-3)] so the DRAM round trips are issued
                # ahead and all three compute engines stay fed.
                for pc in range(NCH2 + 3):
                    if pc < NCH2:
                        stage_gather(pc)
                    if 2 <= pc <= NCH2 + 1:
                        stage_tload(pc - 2)
                    mp = pc - 1 if 1 <= pc <= NCH2 else None
                    op = pc - 3 if pc >= 3 else None
                    if mp is not None or op is not None:
                        stage_mixout(mp, op)
    nc.compile()
    return nc


_NC_CACHE = None


def _get_nc():
    global _NC_CACHE
    if _NC_CACHE is None:
        _NC_CACHE = build_nc()
    return _NC_CACHE


def kernel(in_tensor, w_value, w_key, w_query, w_out, b_out, **_ignored):
    in_tensor = np.asarray(in_tensor, dtype=np.float32)
    w_value = np.asarray(w_value, dtype=np.float32)
    w_key = np.asarray(w_key, dtype=np.float32)
    w_query = np.asarray(w_query, dtype=np.float32)
    w_out = np.asarray(w_out, dtype=np.float32)
    b_out = np.asarray(b_out, dtype=np.float32)

    B = in_tensor.shape[0]
    assert B == 8
    consts = host_constants(w_value, w_key, w_query, w_out, b_out)

    nc = _get_nc()
    in_maps = []
    for b in range(B):
        m = {"x": np.ascontiguousarray(in_tensor[b].reshape(K, C, S))}
        m.update(consts)
        in_maps.append(m)

    from concourse.bass_utils import run_bass_kernel_spmd

    res = run_bass_kernel_spmd(nc, in_maps, core_ids=list(range(8)))
    outs = [
        np.asarray(res.results[b]["out"]).astype(np.float32).reshape(K, C, 64, 64)
        for b in range(B)
    ]
    return np.stack(outs, axis=0)


if __name__ == "__main__":
    build_nc()
    print("built ok")


# revision 31
# speedup vs baseline: 1.1621x; 1.1621x over previous
"""Trainium2 Bass kernel for nn_BaseTransformer (ensemble member-attention block).

Sharding: data-parallel over batch B=8 across 8 NeuronCores (1 batch each).
Weights/constants replicated. No collectives.

Reference math (per batch b, x = in_tensor[b] as [K=16, C=64, S=4096]):
  value = einsum('ics,oc->ios', x, Wv)
  key   = selu(einsum(x, Wk)); query = selu(einsum(x, Wq))
  gram[c,i,j] = sum_s key[i,c,s] query[j,c,s] / 64        (then * lambda^2 fold)
  A = softmax(gram, axis=i) + I
  transformed[j] = sum_i (A[c,i,j] - 1/16) value_i        (exact mean fold)
  out = selu(x + einsum(transformed, w_out) + b_out)

v10 layout/DMA scheme (1126us baseline -> ~387us):
  - x loaded via SWDGE (gpsimd) DMAs with fp32->bf16 cast-in-flight, in two
    s-halves of 16 half tiles [128, 2048] (members t, t+8 stacked).
  - all SBUF->DRAM stores ride SWDGE (descriptors spread over all 16 SDMA
    engines; dynamic HWDGE stores funnel through engines 0/1). Loads stay on
    the sync/scalar HWDGE queues (they spray fine).
  - phase-2 scratch round trips use ONE merged strided DMA per direction per
    chunk (3-dim APs) instead of 8-16 small DMAs.
  - residual add is a PE identity-matmul accumulate from resident bf16 x;
    b_out is folded into the ACT bias of the output selu; output is stored
    bf16 and upcast on host.
  - stride-8 head groups (head c = 8u+g); value/gram q-layout q=64*i2+8u+it
    (pi_in, = baseline) and mix output layout p=64*j2+8*jt+u (pi_out,
    u-innermost) so the mix->outconv regroup is a 3-dim AP.
  - selu(t) = min(alpha*e^t - alpha, relu(t)) composed from ACT Exp,
    ACT Relu, DVE scalar_tensor_tensor.
  - phase 1 software-pipelined (gram skewed one chunk behind kq so the PE
    never waits on the selu chain; value conv spread through the phase);
    phase 2 skewed gather | tload | mix+out so the DRAM round trips hide.
  - kq/kqT storage is g-major (hs = 8g+u <-> head 8u+g) so gram matmul
    operands are contiguous slices (stride-8 APs halve PE throughput).
"""

import sys

if "/opt/trn_rl_repo" not in sys.path:
    sys.path.insert(0, "/opt/trn_rl_repo")

import numpy as np

import concourse.bass as bass
import concourse.bacc as bacc
import concourse.mybir as mybir
import concourse.tile as tile

F32 = mybir.dt.float32
BF16 = mybir.dt.bfloat16

K, C, HEADS, S = 16, 64, 64, 4096
NG = 8           # head groups of 8 (stride-8: group g = heads {8u+g})
SC1 = 128        # phase-1 spatial chunk (gram contraction tile)
NCH1 = S // SC1  # 32
SC2 = 512        # phase-2 spatial chunk
NCH2 = S // SC2  # 8
SH = 1024        # x load quarter
NCH1H = SH // SC1  # 16 phase-1 chunks per half
NCH2H = SH // SC2  # 4 phase-2 chunks per half

ALPHA = 1.6732632423543772
LAMBDA = 1.0507009873554805
LN_ALPHA = float(np.log(ALPHA))
LN_LAMBDA_ALPHA = float(np.log(LAMBDA * ALPHA))
GRAM_SCALE = float(LAMBDA * LAMBDA / 64.0)


def _pi_in(u, i):
    return 64 * (i // 8) + 8 * u + (i % 8)


def _pi_out(u, j):
    return 64 * (j // 8) + 8 * (j % 8) + u


def host_constants(w_value, w_key, w_query, w_out, b_out):
    """Build all replicated device inputs on the host."""
    consts = {}
    # sigma: head c = 8u+g  <->  storage position 8g+u (group-contiguous).
    sigma = np.zeros(64, np.int64)
    for u in range(8):
        for g in range(8):
            sigma[8 * g + u] = 8 * u + g
    wvT = np.ascontiguousarray(w_value.T[:, sigma])
    consts["wvT"] = np.concatenate([wvT, wvT], axis=0).astype(np.float32)
    # k/q head storage is g-major (hs = 8g+u <-> real head 8u+g) so the
    # gram operand APs are contiguous 8-runs instead of stride-8.
    wkqT = np.ascontiguousarray(np.concatenate(
        [w_key.T[:, sigma], w_query.T[:, sigma]], axis=1))
    consts["wkqT"] = np.concatenate([wkqT, wkqT], axis=0).astype(np.float32)

    # Block-diag out-conv weight: W2[(j2', u, g), (j2, o)] = [j2'==j2] *
    # w_out[o, 8u+g]  (tload tile rows are (j2, u, g); po rows are (j2, o)).
    W2 = np.zeros((128, 128), np.float32)
    for j2 in range(2):
        for u in range(8):
            for g in range(8):
                W2[64 * j2 + 8 * u + g, 64 * j2: 64 * (j2 + 1)] = \
                    w_out[:, 8 * u + g]
    consts["W2"] = W2

    # Gram psum layout: partition = 8j+u (q side), free = 8i+u' (k side).
    # MASK zeroes cross-head entries (u != u').
    mask = np.zeros((128, 128), np.float32)
    for p in range(128):
        for f in range(128):
            if p % 8 == f % 8:
                mask[p, f] = 1.0
    consts["maskg"] = mask

    # permP (mm2 lhsT): rows r=(i,u)=8i+u -> bigB row pi_in(u, i).
    P = np.zeros((128, 128), np.float32)
    for u in range(8):
        for i in range(16):
            P[8 * i + u, _pi_in(u, i)] = 1.0
    consts["permP"] = P
    # permPp (mm1 rhs): rows r=(j,u)=8j+u -> bigB column pi_out(u, j).
    Pp = np.zeros((128, 128), np.float32)
    for u in range(8):
        for j in range(16):
            Pp[8 * j + u, _pi_out(u, j)] = 1.0
    consts["permPp"] = Pp

    # DPAT: D[pi_in(u,i), pi_out(u,j)] = delta(i,j) - 1/16.
    D = np.zeros((128, 128), np.float32)
    for u in range(8):
        for i in range(16):
            for j in range(16):
                D[_pi_in(u, i), _pi_out(u, j)] = \
                    (1.0 if i == j else 0.0) - 1.0 / 16.0
    consts["dpat"] = D

    consts["ident"] = np.eye(128, dtype=np.float32)
    # b_out folded into the output selu ACT biases; po rows are (j2, o).
    b2 = np.concatenate([b_out, b_out]).astype(np.float32)
    consts["bo_exp"] = (b2 + LN_LAMBDA_ALPHA).reshape(128, 1)
    consts["bo_relu"] = (b2 * LAMBDA).reshape(128, 1)
    return consts


def build_nc():
    """Build the single-core Bass program (same NEFF on all 8 cores)."""
    nc = bacc.Bacc("TRN2", target_bir_lowering=False, debug=False)

    x_d = nc.dram_tensor("x", [K, C, S], F32, kind="ExternalInput")
    wvT_d = nc.dram_tensor("wvT", [128, 64], F32, kind="ExternalInput")
    wkqT_d = nc.dram_tensor("wkqT", [128, 128], F32, kind="ExternalInput")
    W2_d = nc.dram_tensor("W2", [128, 128], F32, kind="ExternalInput")
    mask_d = nc.dram_tensor("maskg", [128, 128], F32, kind="ExternalInput")
    permP_d = nc.dram_tensor("permP", [128, 128], F32, kind="ExternalInput")
    permPp_d = nc.dram_tensor("permPp", [128, 128], F32, kind="ExternalInput")
    dpat_d = nc.dram_tensor("dpat", [128, 128], F32, kind="ExternalInput")
    ident_d = nc.dram_tensor("ident", [128, 128], F32, kind="ExternalInput")
    boe_d = nc.dram_tensor("bo_exp", [128, 1], F32, kind="ExternalInput")
    bor_d = nc.dram_tensor("bo_relu", [128, 1], F32, kind="ExternalInput")
    out_d = nc.dram_tensor("out", [K, C, S], BF16, kind="ExternalOutput")

    # DRAM scratch for the partition regroups (DRAM-side APs may stride).
    # vscr[pc][g][q=64*i2+8u+it][s]   (value, member-permuted for the mix)
    # mscr[pc][jt][(j2,u,g)][s]       (mix out, regrouped for the out conv)
    vscr_d = nc.dram_tensor("vscr", [NCH2, NG, 128, SC2], BF16)
    mscr_d = nc.dram_tensor("mscr", [NCH2, 8, 128, SC2], BF16)

    with tile.TileContext(nc) as tc:
        with (
            tc.tile_pool(name="persist", bufs=1) as persist,
            tc.tile_pool(name="xpool", bufs=1) as xpool,
        ):
            # ---- weights / constants to SBUF (+ bf16 casts) ----
            wv_f = persist.tile([128, 64], F32, tag="wvf")
            nc.scalar.dma_start(out=wv_f, in_=wvT_d[:, :])
            wv_sb = persist.tile([128, 64], BF16, tag="wv")
            nc.vector.tensor_copy(wv_sb, wv_f)
            wkq_f = persist.tile([128, 128], F32, tag="wkqf")
            nc.scalar.dma_start(out=wkq_f, in_=wkqT_d[:, :])
            wkq_sb = persist.tile([128, 128], BF16, tag="wkq")
            nc.vector.tensor_copy(wkq_sb, wkq_f)
            W2_f = persist.tile([128, 128], F32, tag="W2f")
            nc.scalar.dma_start(out=W2_f, in_=W2_d[:, :])
            W2_sb = persist.tile([128, 128], BF16, tag="W2")
            nc.vector.tensor_copy(W2_sb, W2_f)
            id_f = persist.tile([128, 128], F32, tag="idf")
            nc.scalar.dma_start(out=id_f, in_=ident_d[:, :])
            id_sb = persist.tile([128, 128], BF16, tag="ident")
            nc.vector.tensor_copy(id_sb, id_f)
            mask_sb = persist.tile([128, 128], F32, tag="mask")
            nc.scalar.dma_start(out=mask_sb, in_=mask_d[:, :])
            permP_sb = persist.tile([128, 128], F32, tag="permP")
            nc.scalar.dma_start(out=permP_sb, in_=permP_d[:, :])
            permPp_sb = persist.tile([128, 128], F32, tag="permPp")
            nc.scalar.dma_start(out=permPp_sb, in_=permPp_d[:, :])
            dpat_sb = persist.tile([128, 128], F32, tag="dpat")
            nc.scalar.dma_start(out=dpat_sb, in_=dpat_d[:, :])
            boe_sb = persist.tile([128, 1], F32, tag="boe")
            nc.scalar.dma_start(out=boe_sb, in_=boe_d[:, :])
            bor_sb = persist.tile([128, 1], F32, tag="bor")
            nc.scalar.dma_start(out=bor_sb, in_=bor_d[:, :])
            lna_sb = persist.tile([128, 1], F32, tag="lna")
            nc.vector.memset(lna_sb, LN_ALPHA)
            zero_sb = persist.tile([128, 1], F32, tag="zero")
            nc.vector.memset(zero_sb, 0.0)

            # ---- x: SWDGE casting loads into 16 half tiles [128, 2048] ----
            # x_sb[t][h] holds members (t, t+8), s-range [2048h, 2048h+2048).
            xv_d = x_d.rearrange("(m2 t) c s -> t m2 c s", m2=2, t=8)
            x_sb = [[None] * (S // SH) for _ in range(8)]
            for h in range(S // SH):
                ssl = slice(SH * h, SH * (h + 1))
                for t in range(8):
                    xb = xpool.tile([128, SH], BF16, tag=f"x{t}h{h}")
                    nc.gpsimd.dma_start(out=xb, in_=xv_d[t, :, :, ssl])
                    x_sb[t][h] = xb

            # BigB result tiles (persist into phase 2), bf16 for the mix
            bigB = []
            for g in range(NG):
                bigB_t = persist.tile([128, 128], BF16, tag=f"bigB{g}")
                bigB.append(bigB_t)

            vstores = {}  # pc -> [store instrs]

            # =========================== PHASE 1 ===========================
            # vscr store view: per (pc, i2): [64 p=(g,u)][(t, s) 4096]
            #   dst addr = g*65536 + (64*i2 + 8u + t)*512 + s
            vdst = vscr_d.rearrange(
                "pc g (i2 u it) s -> pc i2 g u (it s)", i2=2, u=8, it=8)
            with (
                tc.tile_pool(name="p1sb", bufs=4) as p1sb,
                tc.tile_pool(name="p1sc", bufs=4) as p1sc,
                tc.tile_pool(name="vall", bufs=3) as vallp,
                tc.tile_pool(name="kqps", bufs=2, space="PSUM") as kqps,
                tc.tile_pool(name="vps", bufs=2, space="PSUM") as vps,
                tc.tile_pool(name="gramps", bufs=1, space="PSUM") as gramps,
            ):
                gram_ps = []
                for gb in range(2):
                    gram_t = gramps.tile([128, 512], F32, tag=f"gram{gb}")
                    gram_ps.append(gram_t)

                kqT_live = {}

                def kq_part(sc):
                    h = sc // NCH1H
                    sl = slice(SC1 * (sc % NCH1H), SC1 * (sc % NCH1H + 1))
                    # kqT free layout: half*1024 + m*64 + c   (bf16)
                    kqT = p1sb.tile([128, K * 128], BF16, tag="kqT")
                    kqT_live[sc] = kqT
                    for blk in range(2):  # member blocks [0..8), [8..16)
                        ps = kqps.tile([128, 8 * 128], F32, tag="kqps")
                        for mb in range(8):
                            m = blk * 8 + mb
                            xt = x_sb[m % 8][h]
                            rhalf = slice(0, 64) if m < 8 else slice(64, 128)
                            nc.tensor.matmul(
                                ps[:, 128 * mb: 128 * (mb + 1)],
                                xt[rhalf, sl], wkq_sb[rhalf, :],
                                start=True, stop=True,
                            )
                        # selu: e2 = exp(kq + ln a); r = relu(kq);
                        # out = (e2 - a) min r   (all bf16 outputs).
                        # e2/r free layout: half*512 + g*64 + mb*8 + u
                        # (g-major, matching the g-major kqT target).
                        e2 = p1sc.tile([128, 8 * 128], BF16, tag="e2")
                        e2v = e2.rearrange("p (h g mb u) -> p mb (h g) u",
                                           h=2, g=8, mb=8, u=8)
                        nc.scalar.activation(
                            out=e2v, in_=ps,
                            func=mybir.ActivationFunctionType.Exp,
                            bias=lna_sb[:, 0:1])
                        r = p1sc.tile([128, 8 * 128], BF16, tag="r")
                        rv = r.rearrange("p (h g mb u) -> p mb (h g) u",
                                         h=2, g=8, mb=8, u=8)
                        nc.scalar.activation(
                            out=rv, in_=ps,
                            func=mybir.ActivationFunctionType.Relu,
                            bias=zero_sb[:, 0:1])
                        stt_eng = nc.vector
                        # kqT free layout: half*1024 + g*128 + m*8 + u
                        # (g-major so the gram operands are contiguous).
                        kqw = kqT.rearrange(
                            "p (hh gg m2 f) -> p hh m2 gg f",
                            hh=2, gg=8, m2=2, f=64)
                        for half in range(2):
                            stt_eng.scalar_tensor_tensor(
                                out=kqw[:, half, blk],
                                in0=e2[:, 512 * half: 512 * (half + 1)],
                                scalar=ALPHA,
                                in1=r[:, 512 * half: 512 * (half + 1)],
                                op0=mybir.AluOpType.subtract,
                                op1=mybir.AluOpType.min)
                def gram_part(sc):
                    # gram: lhsT = q side (M = 8j+u), rhs = k side (N = 8i+u')
                    kqT = kqT_live.pop(sc)
                    vq = kqT.rearrange("p (hh gg f) -> p hh gg f",
                                       hh=2, gg=8, f=128)
                    for g in range(NG):
                        q_ap = vq[:, 1, g]
                        k_ap = vq[:, 0, g]
                        nc.tensor.matmul(
                            gram_ps[g // 4][:, 128 * (g % 4): 128 * (g % 4 + 1)],
                            q_ap, k_ap,
                            start=(sc == 0 and g % 4 == 0),
                            stop=(sc == NCH1 - 1 and g % 4 == 3))

                vall_live = {}

                def value_units(pc, ts):
                    h = pc // NCH2H
                    sl = slice(SC2 * (pc % NCH2H), SC2 * (pc % NCH2H + 1))
                    if pc not in vall_live:
                        vall_new = vallp.tile([128, 8 * SC2], BF16, tag="vall")
                        vall_live[pc] = vall_new
                    vall = vall_live[pc]
                    for t in ts:
                        ps = vps.tile([128, SC2], F32, tag="vps")
                        nc.tensor.matmul(
                            ps[0:64, :], wv_sb[0:64, :], x_sb[t][h][0:64, sl],
                            start=True, stop=True)
                        nc.tensor.matmul(
                            ps[64:128, :], wv_sb[64:128, :],
                            x_sb[t][h][64:128, sl],
                            start=True, stop=True)
                        nc.vector.tensor_copy(
                            vall[:, SC2 * t: SC2 * (t + 1)], ps)
                    if ts[-1] == 7:
                        vall = vall_live.pop(pc)
                        stores = []
                        for i2 in range(2):
                            sti = nc.gpsimd.dma_start(
                                out=vdst[pc, i2],
                                in_=vall[64 * i2: 64 * (i2 + 1), :])
                            stores.append(sti)
                        vstores[pc] = stores

                # Skew gram one chunk behind kq so the PE never waits on the
                # DVE/ACT selu chain (kq(sc+1) fills the gap), and sprinkle
                # value chunks in to keep the PE warm.
                # value-unit schedule: front-loaded (3/chunk after sc 8)
                # so value finishes with the gram; x is fully resident by
                # the time any unit needs a later quarter.
                vsched = [[] for _ in range(NCH1)]
                nxt = 0
                for sc in range(NCH1):
                    take = 2 if sc < 8 else 3
                    for _ in range(take):
                        if nxt < 64:
                            vsched[sc].append((nxt // 8, nxt % 8))
                            nxt += 1
                for sc in range(NCH1):
                    kq_part(sc)
                    if sc >= 1:
                        gram_part(sc - 1)
                    for pc, t in vsched[sc]:
                        value_units(pc, [t])
                gram_part(NCH1 - 1)

                # ---- softmax (no max-sub; range pre-verified) + BigB ----
                for g in range(NG):
                    gp = gram_ps[g // 4][:, 128 * (g % 4): 128 * (g % 4 + 1)]
                    E = p1sc.tile([128, 128], F32, tag="E")
                    nc.scalar.activation(
                        out=E, in_=gp,
                        func=mybir.ActivationFunctionType.Exp,
                        bias=zero_sb[:, 0:1], scale=GRAM_SCALE)
                    Ssum = p1sc.tile([128, 8], F32, tag="Ssum")
                    nc.vector.tensor_reduce(
                        out=Ssum,
                        in_=E.rearrange("p (i u) -> p u i", i=16, u=8),
                        axis=mybir.AxisListType.X, op=mybir.AluOpType.add)
                    R = p1sc.tile([128, 8], F32, tag="R")
                    nc.vector.reciprocal(out=R, in_=Ssum)
                    Eu = E.rearrange("p (i u) -> p u i", i=16, u=8)
                    for u in range(8):
                        nc.vector.tensor_scalar(
                            out=Eu[:, u, :], in0=Eu[:, u, :],
                            scalar1=R[:, u: u + 1], scalar2=None,
                            op0=mybir.AluOpType.mult)
                    nc.vector.tensor_tensor(
                        out=E, in0=E, in1=mask_sb, op=mybir.AluOpType.mult)
                    c_ps = kqps.tile([128, 128], F32, tag="kqps")
                    nc.tensor.matmul(c_ps, E, permPp_sb, start=True, stop=True)
                    c_sb = p1sc.tile([128, 128], F32, tag="permcsb")
                    nc.scalar.copy(c_sb, c_ps)
                    b_ps = kqps.tile([128, 128], F32, tag="kqps")
                    nc.tensor.matmul(b_ps, permP_sb, c_sb, start=True, stop=True)
                    nc.vector.scalar_tensor_tensor(
                        out=bigB[g], in0=b_ps, scalar=1.0, in1=dpat_sb,
                        op0=mybir.AluOpType.mult, op1=mybir.AluOpType.add)

            # =========================== PHASE 2 ===========================
            # Per chunk: merged vgather -> 8 mix matmuls -> merged mstore ->
            # merged tload -> per member-pair: out conv + identity-residual
            # matmul, selu via ACT Exp/Relu (b_out in bias) + DVE stt ->
            # merged bf16 outstore.
            vgat = vscr_d.rearrange("pc g q s -> pc q g s")
            # mscr store view: per (pc, j2): [64 p=(jt,u)][(g, s) 4096]
            mdst = mscr_d.rearrange(
                "pc jt (j2 u g) s -> pc j2 jt u (g s)", j2=2, u=8, g=8)
            mgat = mscr_d.rearrange("pc jt q s -> pc q jt s")
            # out store view: per (pc, j2): [64 p=o][(jt, s)]
            odst = out_d.rearrange("(j2 jt) c s -> j2 c jt s", j2=2, jt=8)
            with (
                tc.tile_pool(name="p2in", bufs=4) as p2in,
                tc.tile_pool(name="p2sc", bufs=3) as p2sc,
                tc.tile_pool(name="p2out", bufs=2) as p2outp,
                tc.tile_pool(name="mps", bufs=5, space="PSUM") as mps,
                tc.tile_pool(name="ops", bufs=3, space="PSUM") as ops,
            ):
                mstores = {}
                vgs = {}
                talls = {}

                def stage_gather(pc):
                    vg = p2in.tile([128, 8 * SC2], BF16, tag="vg")
                    gi = nc.sync.dma_start(out=vg, in_=vgat[pc])
                    for sti in vstores.pop(pc):
                        tile.add_dep_helper(
                            gi.ins, sti.ins, reason="vgather after vstores")
                    vgs[pc] = vg

                def stage_mixout(mp, op):
                    """Interleave mix units of chunk mp with out units of
                    chunk op so PE/ACT/DVE all stay fed."""
                    vg = vgs.pop(mp) if mp is not None else None
                    mall = None
                    if mp is not None:
                        mall = p2sc.tile([128, 8 * SC2], BF16, tag="mall")
                    if op is not None:
                        h = op // NCH2H
                        sl = slice(SC2 * (op % NCH2H), SC2 * (op % NCH2H + 1))
                        gsl = slice(SC2 * op, SC2 * (op + 1))
                        tall = talls.pop(op)
                        o_all = p2outp.tile([128, 8 * SC2], BF16, tag="oall")
                    for k in range(8):
                        if op is not None:
                            po = ops.tile([128, SC2], F32, tag="ops")
                            nc.tensor.matmul(
                                po, W2_sb, tall[:, SC2 * k: SC2 * (k + 1)],
                                start=True, stop=False)
                            nc.tensor.matmul(
                                po, id_sb, x_sb[k][h][:, sl],
                                start=False, stop=True)
                            e2f = p2sc.tile([128, SC2], BF16, tag="fe2")
                            nc.scalar.activation(
                                out=e2f, in_=po,
                                func=mybir.ActivationFunctionType.Exp,
                                bias=boe_sb[:, 0:1])
                            r2f = p2sc.tile([128, SC2], BF16, tag="fr2")
                            nc.scalar.activation(
                                out=r2f, in_=po,
                                func=mybir.ActivationFunctionType.Relu,
                                bias=bor_sb[:, 0:1], scale=LAMBDA)
                            nc.vector.scalar_tensor_tensor(
                                out=o_all[:, SC2 * k: SC2 * (k + 1)],
                                in0=e2f, scalar=float(LAMBDA * ALPHA),
                                in1=r2f,
                                op0=mybir.AluOpType.subtract,
                                op1=mybir.AluOpType.min)
                    for k in range(8):
                        if mp is not None:
                            pm = mps.tile([128, SC2], F32, tag="mps")
                            nc.tensor.matmul(
                                pm, bigB[k], vg[:, SC2 * k: SC2 * (k + 1)],
                                start=True, stop=True)
                            # drain copies alternate DVE/ACT: the mix phase
                            # is paced by these, and ACT is idle during it
                            if k % 2 == 0:
                                nc.vector.tensor_copy(
                                    mall[:, SC2 * k: SC2 * (k + 1)], pm)
                            else:
                                nc.scalar.copy(
                                    mall[:, SC2 * k: SC2 * (k + 1)], pm)
                    if mp is not None:
                        mst = []
                        for j2 in range(2):
                            si = nc.gpsimd.dma_start(
                                out=mdst[mp, j2],
                                in_=mall[64 * j2: 64 * (j2 + 1), :])
                            mst.append(si)
                        mstores[mp] = mst
                    if op is not None:
                        for j2 in range(2):
                            nc.gpsimd.dma_start(
                                out=odst[j2, :, :, gsl],
                                in_=o_all[64 * j2: 64 * (j2 + 1), :])

                def stage_tload(pc):
                    tall = p2in.tile([128, 8 * SC2], BF16, tag="tall")
                    li = nc.scalar.dma_start(out=tall, in_=mgat[pc])
                    for si in mstores.pop(pc):
                        tile.add_dep_helper(
                            li.ins, si.ins, reason="tload after mstores")
                    talls[pc] = tall

                # Skewed pipeline: gather(pc) | tload(pc-2) | interleaved
                # [mix(pc-1) + out(pc-3)] so the DRAM round trips are issued
                # ahead and all three compute engines stay fed.
                for pc in range(NCH2 + 3):
                    if pc < NCH2:
                        stage_gather(pc)
                    if 2 <= pc <= NCH2 + 1:
                        stage_tload(pc - 2)
                    mp = pc - 1 if 1 <= pc <= NCH2 else None
                    op = pc - 3 if pc >= 3 else None
                    if mp is not None or op is not None:
                        stage_mixout(mp, op)
    nc.compile()
    return nc


_NC_CACHE = None


def _get_nc():
    global _NC_CACHE
    if _NC_CACHE is None:
        _NC_CACHE = build_nc()
    return _NC_CACHE


def kernel(in_tensor, w_value, w_key, w_query, w_out, b_out, **_ignored):
    in_tensor = np.asarray(in_tensor, dtype=np.float32)
    w_value = np.asarray(w_value, dtype=np.float32)
    w_key = np.asarray(w_key, dtype=np.float32)
    w_query = np.asarray(w_query, dtype=np.float32)
    w_out = np.asarray(w_out, dtype=np.float32)
    b_out = np.asarray(b_out, dtype=np.float32)

    B = in_tensor.shape[0]
    assert B == 8
    consts = host_constants(w_value, w_key, w_query, w_out, b_out)

    nc = _get_nc()
    in_maps = []
    for b in range(B):
        m = {"x": np.ascontiguousarray(in_tensor[b].reshape(K, C, S))}
        m.update(consts)
        in_maps.append(m)

    from concourse.bass_utils import run_bass_kernel_spmd

    res = run_bass_kernel_spmd(nc, in_maps, core_ids=list(range(8)))
    outs = [
        np.asarray(res.results[b]["out"]).astype(np.float32).reshape(K, C, 64, 64)
        for b in range(B)
    ]
    return np.stack(outs, axis=0)


if __name__ == "__main__":
    build_nc()
    print("built ok")
